# revision 4
# baseline (speedup 1.0000x reference)
"""Trainium2 Bass kernel for nn_DynamicHead (varying-coefficient spline MLP).

Math: basis(t) = [1,t,t^2,t^3, relu(t-k_j)^3 ...] (12 fns, 8 knots at j/9).
Each vc_layer: out = sum_s basis_s * (x @ W_s) + basis @ b.
Within knot segment m (t in [k_m, k_{m+1})), relu terms are plain cubics, so
basis collapses to powers [1,t,t^2,t^3] against segment-combined weights
C[m,p] = sum_s gamma[m,p,s] W_s.  Each layer is then a K=4*256 matmul over
z[(p,i), b] = t^p * x[i, b] for samples grouped by segment.

Host: sort samples by segment, deal round-robin across 8 cores (uniform
segment sizes), pad each per-core segment to CAP. Device (per core, SPMD):
for each segment, 3 layers of fp32r matmuls (feature-major activations),
DVE builds z tiles from DMA-broadcast power rows, ScalarE does relu+PSUM
evacuation. Final layer (out_dim=1) contracts features first (q = C2 @ x),
then basis via a 4-partition elementwise + ones-matmul reduction.
"""
import sys
import types

for _p in ('/opt/trn_rl_repo', '/root/.axon_site/_ro/trn_rl_repo'):
    if _p not in sys.path:
        sys.path.append(_p)

import numpy as np
import concourse.bass as bass
import concourse.tile as tile
from concourse import bacc, mybir
from concourse import bass_utils

F32 = mybir.dt.float32
F32R = mybir.dt.float32r
RELU = mybir.ActivationFunctionType.Relu
COPY = mybir.ActivationFunctionType.Copy

B, D, NSEG, NSB = 32768, 256, 9, 4
N_CORES = 8
BC = B // N_CORES                      # samples per core
KNOTS = np.array([i / 9.0 for i in range(1, 9)], dtype=np.float64)
SDIM = 12

# set True by test harness for a profiled run
TRACE = False
LAST_EXEC_NS = None
LAST_MEAN_EXEC_NS = None
LAST_RES = None

_PROG_CACHE = {}


def _register_ntff_hook():
    try:
        import antenv.axon_hooks  # noqa: F401
        return
    except ImportError:
        pass
    try:
        from trn_agent_boot.trn_boot import _ntff_profile_via_ctypes
        hook = _ntff_profile_via_ctypes('/opt/axon/libaxon_pjrt.so')
        mod = types.ModuleType('antenv.axon_hooks')
        mod.get_axon_ntff_profile_hook = lambda: hook
        sys.modules['antenv.axon_hooks'] = mod
    except Exception:
        pass


def _gamma() -> np.ndarray:
    """(NSEG, NSB, SDIM): basis -> per-segment cubic coefficients."""
    g = np.zeros((NSEG, NSB, SDIM), dtype=np.float64)
    for m in range(NSEG):
        for p in range(NSB):
            g[m, p, p] = 1.0
        for j in range(1, 9):          # spline s = 3 + j, knot k = j/9
            if j <= m:
                k = KNOTS[j - 1]
                g[m, 0, 3 + j] = -k ** 3
                g[m, 1, 3 + j] = 3 * k ** 2
                g[m, 2, 3 + j] = -3 * k
                g[m, 3, 3 + j] = 1.0
    return g


def _chunks(cap):
    """Split a segment capacity into matmul-N chunks (<=512 each)."""
    out, off = [], 0
    while off < cap:
        n = min(512, cap - off)
        out.append((off, n))
        off += n
    return out


def _build_program(cap):
    """Build + compile the SPMD single-core program for per-segment capacity cap."""
    bp = cap * NSEG                     # padded per-core batch
    nc = bacc.Bacc("TRN2", target_bir_lowering=False, debug=False,
                   num_devices=N_CORES)

    xT_ap = nc.dram_tensor("xT", [D, bp], F32, kind="ExternalInput").ap()
    tp_ap = nc.dram_tensor("tp", [NSB, bp], F32, kind="ExternalInput").ap()
    c0w_ap = nc.dram_tensor("c0w", [NSEG, NSB * D, D], F32, kind="ExternalInput").ap()
    c0b_ap = nc.dram_tensor("c0b", [NSEG, NSB, D], F32, kind="ExternalInput").ap()
    c1w_ap = nc.dram_tensor("c1w", [NSEG, NSB * D, D], F32, kind="ExternalInput").ap()
    c1b_ap = nc.dram_tensor("c1b", [NSEG, NSB, D], F32, kind="ExternalInput").ap()
    c2w_ap = nc.dram_tensor("c2w", [NSEG, D, NSB], F32, kind="ExternalInput").ap()
    c2b_ap = nc.dram_tensor("c2b", [NSEG, NSB], F32, kind="ExternalInput").ap()
    ones_ap = nc.dram_tensor("ones4", [NSB, 1], F32, kind="ExternalInput").ap()
    out_ap = nc.dram_tensor("out", [1, bp], F32, kind="ExternalOutput").ap()

    NKT = NSB * D // 128               # 8 k-tiles of 128
    cw_ap = (c0w_ap, c1w_ap)
    cb_ap = (c0b_ap, c1b_ap)

    with tile.TileContext(nc) as tc:
        with (
            tc.tile_pool(name="act", bufs=1) as actp,
            tc.tile_pool(name="bc", bufs=1) as bcp,
            tc.tile_pool(name="z", bufs=1) as zp,
            tc.tile_pool(name="w", bufs=1) as wp,
            tc.tile_pool(name="sm", bufs=1) as smp,
            tc.tile_pool(name="pm", bufs=1, space="PSUM") as pmp,
            tc.tile_pool(name="pq", bufs=1, space="PSUM") as pqp,
        ):
            ones4 = smp.tile([NSB, 1], F32R, name="ones4", tag="ones4")
            nc.sync.dma_start(ones4[:, :], ones_ap[:, :].bitcast(F32R))

            # per-segment state, created by stage functions
            xin, x1, x2, bcast, tps = {}, {}, {}, {}, {}

            def load_seg(s):
                sl = slice(s * cap, (s + 1) * cap)
                xin[s] = [actp.tile([128, cap], F32R, name=f"xin{s}_{h}",
                                    tag=f"xin{h}", bufs=3) for h in range(2)]
                for h in range(2):
                    nc.sync.dma_start(xin[s][h][:, :],
                                      xT_ap[h * 128:(h + 1) * 128, sl].bitcast(F32R))
                bcast[s] = bcp.tile([128, (NSB - 1) * cap], F32R,
                                    name=f"bc{s}", tag="bc", bufs=3)
                for p in range(1, NSB):
                    nc.sync.dma_start(
                        bcast[s][:, (p - 1) * cap:p * cap],
                        tp_ap[p, sl].partition_broadcast(128).bitcast(F32R))
                tps[s] = smp.tile([NSB, cap], F32R, name=f"tp{s}", tag="tp", bufs=3)
                nc.sync.dma_start(tps[s][:, :], tp_ap[:, sl].bitcast(F32R))

            def vc_layer(s, L, xin_t, store):
                """layers 0/1: (o,b) = relu(C.T @ z + Cb.T @ tp), feature-major"""
                wt = wp.tile([128, NKT * D], F32R, name=f"w{L}_{s}", tag="w", bufs=4)
                nc.sync.dma_start(
                    wt[:, :].rearrange("p (kt o) -> p kt o", kt=NKT),
                    cw_ap[L][s].rearrange("(kt p) o -> p kt o", p=128).bitcast(F32R))
                wb = wp.tile([NSB, D], F32R, name=f"wb{L}_{s}", tag="wb", bufs=4)
                nc.sync.dma_start(wb[:, :], cb_ap[L][s].bitcast(F32R))

                zt = zp.tile([128, (NSB - 1) * 2 * cap], F32R,
                             name=f"z{L}_{s}", tag="z", bufs=3)
                for p in range(1, NSB):
                    for h in range(2):
                        blk = (p - 1) * 2 + h
                        nc.vector.tensor_mul(
                            zt[:, blk * cap:(blk + 1) * cap],
                            xin_t[h][:, :],
                            bcast[s][:, (p - 1) * cap:p * cap])

                outs = []
                for m in range(2):
                    for off, n in _chunks(cap):
                        ps = pmp.tile([128, n], F32, name=f"pm{L}_{s}_{m}_{off}",
                                      tag="pm", bufs=4)
                        for kt in range(NKT):
                            p, h = divmod(kt, 2)
                            if p == 0:
                                rhs = xin_t[h][:, off:off + n]
                            else:
                                blk = (p - 1) * 2 + h
                                rhs = zt[:, blk * cap + off:blk * cap + off + n]
                            nc.tensor.matmul(
                                ps[:, :],
                                wt[:, kt * D + m * 128:kt * D + (m + 1) * 128],
                                rhs, start=(kt == 0), stop=False)
                        nc.tensor.matmul(ps[:, :], wb[:, m * 128:(m + 1) * 128],
                                         tps[s][:, off:off + n],
                                         start=False, stop=True)
                        outs.append((m, off, n, ps))
                xo = [actp.tile([128, cap], F32R, name=f"x{L + 1}_{s}_{h}",
                                tag=f"xo{L}{h}", bufs=3) for h in range(2)]
                for m, off, n, ps in outs:
                    nc.scalar.activation(xo[m][:, off:off + n], ps[:, :], RELU)
                store[s] = xo

            def head_layer(s):
                """layer 2 (out_dim=1): q=C2.T@x2 (+b2), out = ones.T @ (q*tp)"""
                sl = slice(s * cap, (s + 1) * cap)
                c2w = smp.tile([128, 2 * NSB], F32R, name=f"c2w{s}", tag="c2w", bufs=3)
                nc.sync.dma_start(
                    c2w[:, :].rearrange("p (h q) -> p h q", h=2),
                    c2w_ap[s].rearrange("(h p) q -> p h q", p=128).bitcast(F32R))
                c2b = smp.tile([NSB, 1], F32, name=f"c2b{s}", tag="c2b", bufs=3)
                nc.sync.dma_start(
                    c2b[:, :], c2b_ap[s:s + 1, :].rearrange("a b -> b a"))
                for off, n in _chunks(cap):
                    psq = pqp.tile([NSB, n], F32, name=f"pq{s}_{off}", tag="pq", bufs=2)
                    for h in range(2):
                        nc.tensor.matmul(psq[:, :],
                                         c2w[:, h * NSB:(h + 1) * NSB],
                                         x2[s][h][:, off:off + n],
                                         start=(h == 0), stop=(h == 1))
                    qb = smp.tile([NSB, n], F32R, name=f"qb{s}_{off}", tag="qb", bufs=3)
                    nc.scalar.activation(qb[:, :], psq[:, :],
                                         mybir.ActivationFunctionType.Identity,
                                         bias=c2b[:, :])
                    rq = smp.tile([NSB, n], F32R, name=f"rq{s}_{off}", tag="rq", bufs=3)
                    nc.vector.tensor_mul(rq[:, :], qb[:, :],
                                         tps[s][:, off:off + n])
                    psr = pqp.tile([1, n], F32, name=f"pr{s}_{off}", tag="pr", bufs=2)
                    nc.tensor.matmul(psr[:, :], ones4[:, :], rq[:, :],
                                     start=True, stop=True)
                    orow = smp.tile([1, n], F32, name=f"or{s}_{off}", tag="or", bufs=3)
                    nc.scalar.activation(orow[:, :], psr[:, :], COPY)
                    nc.sync.dma_start(out_ap[0:1, s * cap + off:s * cap + off + n],
                                      orow[:, :])

            # segment-skewed software pipeline: L0(s+1) overlaps L1(s)/L2(s-1)
            for step in range(NSEG + 2):
                if step < NSEG:
                    load_seg(step)
                    vc_layer(step, 0, xin[step], x1)
                    xin.pop(step)
                if 1 <= step < NSEG + 1:
                    vc_layer(step - 1, 1, x1[step - 1], x2)
                    x1.pop(step - 1)
                if step >= 2:
                    head_layer(step - 2)
                    x2.pop(step - 2)

    nc.compile()
    return nc


def _prep_host(treatment, features, W0, b0, W1, b1, W2, b2):
    t = np.asarray(treatment, dtype=np.float32)
    x = np.asarray(features, dtype=np.float32)
    seg = np.searchsorted(KNOTS.astype(np.float32), t, side='right')

    # deal each segment round-robin across cores
    core_of = np.empty(B, dtype=np.int64)
    pos_of = np.empty(B, dtype=np.int64)     # slot within (core, segment)
    counts = np.zeros((N_CORES, NSEG), dtype=np.int64)
    for m in range(NSEG):
        idx = np.nonzero(seg == m)[0]
        for c in range(N_CORES):
            sub = idx[c::N_CORES]
            core_of[sub] = c
            pos_of[sub] = np.arange(len(sub))
            counts[c, m] = len(sub)
    maxn = int(counts.max())
    cap = max(512, ((maxn + 127) // 128) * 128)
    bp = cap * NSEG

    # per-core padded gather index (-1 = pad)
    gather = np.full((N_CORES, bp), -1, dtype=np.int64)
    slot = seg * cap + pos_of
    gather[core_of, slot] = np.arange(B)

    xT = np.zeros((N_CORES, D, bp), dtype=np.float32)
    tp = np.zeros((N_CORES, NSB, bp), dtype=np.float32)
    for c in range(N_CORES):
        v = gather[c] >= 0
        gi = gather[c][v]
        xT[c][:, v] = x[gi].T
        tv = t[gi].astype(np.float64)
        tp[c][:, v] = np.stack([tv ** p for p in range(NSB)]).astype(np.float32)

    g = _gamma()
    W0s = np.asarray(W0, dtype=np.float64).reshape(SDIM, D, D)
    W1s = np.asarray(W1, dtype=np.float64).reshape(SDIM, D, D)
    c0w = np.einsum('mps,sio->mpio', g, W0s).reshape(NSEG, NSB * D, D).astype(np.float32)
    c1w = np.einsum('mps,sio->mpio', g, W1s).reshape(NSEG, NSB * D, D).astype(np.float32)
    c0b = np.einsum('mps,so->mpo', g, np.asarray(b0, np.float64)).astype(np.float32)
    c1b = np.einsum('mps,so->mpo', g, np.asarray(b1, np.float64)).astype(np.float32)
    c2w = np.einsum('mps,si->mip', g, np.asarray(W2, np.float64)).astype(np.float32)
    c2b = np.einsum('mps,s->mp', g, np.asarray(b2, np.float64)[:, 0]).astype(np.float32)

    shared = dict(c0w=np.ascontiguousarray(c0w), c0b=np.ascontiguousarray(c0b),
                  c1w=np.ascontiguousarray(c1w), c1b=np.ascontiguousarray(c1b),
                  c2w=np.ascontiguousarray(c2w), c2b=np.ascontiguousarray(c2b),
                  ones4=np.ones((NSB, 1), np.float32))
    in_maps = [dict(shared, xT=np.ascontiguousarray(xT[c]),
                    tp=np.ascontiguousarray(tp[c])) for c in range(N_CORES)]
    return cap, in_maps, gather


def kernel(treatment, features, W0, b0, W1, b1, W2, b2):
    global LAST_EXEC_NS, LAST_MEAN_EXEC_NS, LAST_RES
    cap, in_maps, gather = _prep_host(treatment, features, W0, b0, W1, b1, W2, b2)

    if cap not in _PROG_CACHE:
        _PROG_CACHE[cap] = _build_program(cap)
    nc = _PROG_CACHE[cap]

    if TRACE:
        _register_ntff_hook()
    res = bass_utils.run_bass_kernel_spmd(
        nc, in_maps, core_ids=list(range(N_CORES)), trace=TRACE)
    LAST_EXEC_NS = res.exec_time_ns
    LAST_MEAN_EXEC_NS = res.mean_exec_time_ns
    LAST_RES = res

    out = np.empty((B,), dtype=np.float32)
    for c in range(N_CORES):
        row = res.results[c]["out"][0]
        v = gather[c] >= 0
        out[gather[c][v]] = row[v]
    return out.reshape(B, 1)


# revision 7
# speedup vs baseline: 1.0965x; 1.0965x over previous
"""Trainium2 Bass kernel for nn_DynamicHead (varying-coefficient spline MLP).

Math: basis(t) = [1,t,t^2,t^3, relu(t-k_j)^3 ...] (12 fns, 8 knots at j/9).
Each vc_layer: out = sum_s basis_s * (x @ W_s) + basis @ b.
Within knot segment m (t in [k_m, k_{m+1})), relu terms are plain cubics, so
basis collapses to powers [1,t,t^2,t^3] against segment-combined weights
C[m,p] = sum_s gamma[m,p,s] W_s.  Each layer is then a K=4*256 matmul over
z[(p,i), b] = t^p * x[i, b] for samples grouped by segment.

Host: sort samples by segment, deal round-robin across 8 cores (uniform
segment sizes), pad each per-core segment to CAP; prepack weights into the
exact SBUF tile layouts. Device (per core, SPMD): for each segment, 3 layers
of fp32r matmuls (feature-major activations), DVE builds z tiles from
DMA-broadcast power rows, ScalarE does relu+PSUM evacuation; final layer
(out_dim=1) contracts features first, then basis via a 4-partition
elementwise + ones-matmul reduction. Segment-skewed software pipeline.
"""
import os
import sys
import types

for _p in ('/opt/trn_rl_repo', '/root/.axon_site/_ro/trn_rl_repo'):
    if _p not in sys.path:
        sys.path.append(_p)

import numpy as np
import concourse.bass as bass
import concourse.tile as tile
from concourse import bacc, mybir
from concourse import bass_utils

F32 = mybir.dt.float32
F32R = mybir.dt.float32r
RELU = mybir.ActivationFunctionType.Relu
COPY = mybir.ActivationFunctionType.Copy
IDENT = mybir.ActivationFunctionType.Identity

B, D, NSEG, NSB = 32768, 256, 9, 4
N_CORES = 8
KNOTS = np.array([i / 9.0 for i in range(1, 9)], dtype=np.float64)
SDIM = 12
NKT = NSB * D // 128                   # 8 k-tiles of 128

# set True by test harness for a profiled run
TRACE = False
LAST_EXEC_NS = None
LAST_MEAN_EXEC_NS = None
LAST_RES = None

_PROG_CACHE = {}

if os.environ.get("BASS_LDW_OPT") == "1":
    _orig_run_command = bass_utils.run_command

    def _run_command_ldw(argv, **kw):
        argv = ["--enable-ldw-opt=true" if a == "--enable-ldw-opt=false" else a
                for a in argv]
        return _orig_run_command(argv, **kw)

    bass_utils.run_command = _run_command_ldw


def _register_ntff_hook():
    try:
        import antenv.axon_hooks  # noqa: F401
        return
    except ImportError:
        pass
    try:
        from trn_agent_boot.trn_boot import _ntff_profile_via_ctypes
        hook = _ntff_profile_via_ctypes('/opt/axon/libaxon_pjrt.so')
        mod = types.ModuleType('antenv.axon_hooks')
        mod.get_axon_ntff_profile_hook = lambda: hook
        sys.modules['antenv.axon_hooks'] = mod
    except Exception:
        pass


def _gamma() -> np.ndarray:
    """(NSEG, NSB, SDIM): basis -> per-segment cubic coefficients."""
    g = np.zeros((NSEG, NSB, SDIM), dtype=np.float64)
    for m in range(NSEG):
        for p in range(NSB):
            g[m, p, p] = 1.0
        for j in range(1, 9):          # spline s = 3 + j, knot k = j/9
            if j <= m:
                k = KNOTS[j - 1]
                g[m, 0, 3 + j] = -k ** 3
                g[m, 1, 3 + j] = 3 * k ** 2
                g[m, 2, 3 + j] = -3 * k
                g[m, 3, 3 + j] = 1.0
    return g


def _chunks(cap):
    out, off = [], 0
    while off < cap:
        n = min(512, cap - off)
        out.append((off, n))
        off += n
    return out


def _build_program(cap):
    """Build + compile the SPMD single-core program for per-segment capacity cap."""
    bp = cap * NSEG                     # padded per-core batch
    nc = bacc.Bacc("TRN2", target_bir_lowering=False, debug=False,
                   num_devices=N_CORES)

    # cw: per (layer, seg) prepacked (128, 2304): 8 k-tile blocks of 256 (o)
    # cols + bias block (partitions 0..3) at cols 2048..2303.
    xT_ap = nc.dram_tensor("xT", [D, bp], F32, kind="ExternalInput").ap()
    tp_ap = nc.dram_tensor("tp", [NSB, bp], F32, kind="ExternalInput").ap()
    c0w_ap = nc.dram_tensor("c0w", [NSEG, 128, (NKT + 1) * D], F32, kind="ExternalInput").ap()
    c1w_ap = nc.dram_tensor("c1w", [NSEG, 128, (NKT + 1) * D], F32, kind="ExternalInput").ap()
    # c2: prepacked (128, 9): cols h*4..h*4+3 = c2w k-tile h, col 8 (parts 0..3) = bias
    c2_ap = nc.dram_tensor("c2", [NSEG, 128, 2 * NSB + 1], F32, kind="ExternalInput").ap()
    ones_ap = nc.dram_tensor("ones4", [NSB, 1], F32, kind="ExternalInput").ap()
    out_ap = nc.dram_tensor("out", [1, bp], F32, kind="ExternalOutput").ap()

    cw_ap = (c0w_ap, c1w_ap)

    with tile.TileContext(nc) as tc:
        with (
            tc.tile_pool(name="act", bufs=1) as actp,
            tc.tile_pool(name="bc", bufs=1) as bcp,
            tc.tile_pool(name="z", bufs=1) as zp,
            tc.tile_pool(name="w", bufs=1) as wp,
            tc.tile_pool(name="sm", bufs=1) as smp,
            tc.tile_pool(name="pm", bufs=1, space="PSUM") as pmp,
            tc.tile_pool(name="pq", bufs=1, space="PSUM") as pqp,
        ):
            ones4 = smp.tile([NSB, 1], F32R, name="ones4", tag="ones4")
            nc.gpsimd.dma_start(ones4[:, :], ones_ap[:, :].bitcast(F32R))

            xin, x1, x2, bcast, tps = {}, {}, {}, {}, {}

            def load_seg(s):
                sl = slice(s * cap, (s + 1) * cap)
                # both feature halves in one tile: cols h*cap..(h+1)*cap
                xt = actp.tile([128, 2 * cap], F32R, name=f"xin{s}",
                               tag="xin", bufs=3)
                nc.gpsimd.dma_start(
                    xt[:, :].rearrange("p (h b) -> p h b", h=2),
                    xT_ap[:, sl].rearrange("(h p) b -> p h b", p=128).bitcast(F32R))
                xin[s] = xt
                # power rows broadcast to 128 partitions, h-doubled:
                # block (p-1): [bc_p | bc_p] each of width cap
                bt = bcp.tile([128, (NSB - 1) * 2 * cap], F32R,
                              name=f"bc{s}", tag="bc", bufs=3)
                bt4 = bt[:, :].rearrange("q (p h b) -> q p h b", p=NSB - 1, h=2)
                for h in range(2):
                    nc.gpsimd.dma_start(
                        bt4[:, :, h, :],
                        tp_ap[1:NSB, sl].partition_broadcast(128).bitcast(F32R))
                bcast[s] = bt
                tps[s] = smp.tile([NSB, cap], F32R, name=f"tp{s}", tag="tp", bufs=3)
                nc.gpsimd.dma_start(tps[s][:, :], tp_ap[:, sl].bitcast(F32R))

            def vc_layer(s, L, xin_t, store):
                """layers 0/1: (o,b) = relu(C.T @ z + Cb.T @ tp), feature-major"""
                wt = wp.tile([128, (NKT + 1) * D], F32R, name=f"w{L}_{s}",
                             tag="w", bufs=4)
                nc.sync.dma_start(wt[:, :], cw_ap[L][s].bitcast(F32R))

                zt = zp.tile([128, (NSB - 1) * 2 * cap], F32R,
                             name=f"z{L}_{s}", tag="z", bufs=3)
                for p in range(1, NSB):
                    blk = (p - 1) * 2 * cap
                    nc.vector.tensor_mul(
                        zt[:, blk:blk + 2 * cap],
                        xin_t[:, :],
                        bcast[s][:, blk:blk + 2 * cap])

                outs = []
                for m in range(2):
                    for off, n in _chunks(cap):
                        ps = pmp.tile([128, n], F32, name=f"pm{L}_{s}_{m}_{off}",
                                      tag="pm", bufs=4)
                        for kt in range(NKT):
                            p, h = divmod(kt, 2)
                            if p == 0:
                                rhs = xin_t[:, h * cap + off:h * cap + off + n]
                            else:
                                blk = ((p - 1) * 2 + h) * cap
                                rhs = zt[:, blk + off:blk + off + n]
                            nc.tensor.matmul(
                                ps[:, :],
                                wt[:, kt * D + m * 128:kt * D + (m + 1) * 128],
                                rhs, start=(kt == 0), stop=False)
                        nc.tensor.matmul(ps[:, :],
                                         wt[0:NSB, NKT * D + m * 128:NKT * D + (m + 1) * 128],
                                         tps[s][:, off:off + n],
                                         start=False, stop=True)
                        outs.append((m, off, n, ps))
                xo = actp.tile([128, 2 * cap], F32R, name=f"x{L + 1}_{s}",
                               tag=f"xo{L}", bufs=3)
                for m, off, n, ps in outs:
                    nc.scalar.activation(xo[:, m * cap + off:m * cap + off + n],
                                         ps[:, :], RELU)
                store[s] = xo

            def head_layer(s):
                """layer 2 (out_dim=1): q=C2.T@x2 (+b2), out = ones.T @ (q*tp)"""
                c2t = smp.tile([128, 2 * NSB + 1], F32R, name=f"c2_{s}",
                               tag="c2", bufs=3)
                nc.gpsimd.dma_start(c2t[:, :], c2_ap[s].bitcast(F32R))
                for off, n in _chunks(cap):
                    psq = pqp.tile([NSB, n], F32, name=f"pq{s}_{off}", tag="pq", bufs=2)
                    for h in range(2):
                        nc.tensor.matmul(psq[:, :],
                                         c2t[:, h * NSB:(h + 1) * NSB],
                                         x2[s][:, h * cap + off:h * cap + off + n],
                                         start=(h == 0), stop=(h == 1))
                    qb = smp.tile([NSB, n], F32R, name=f"qb{s}_{off}", tag="qb", bufs=3)
                    nc.scalar.activation(qb[:, :], psq[:, :], IDENT,
                                         bias=c2t[0:NSB, 2 * NSB:2 * NSB + 1].bitcast(F32))
                    rq = smp.tile([NSB, n], F32R, name=f"rq{s}_{off}", tag="rq", bufs=3)
                    nc.vector.tensor_mul(rq[:, :], qb[:, :],
                                         tps[s][:, off:off + n])
                    psr = pqp.tile([1, n], F32, name=f"pr{s}_{off}", tag="pr", bufs=2)
                    nc.tensor.matmul(psr[:, :], ones4[:, :], rq[:, :],
                                     start=True, stop=True)
                    orow = smp.tile([1, n], F32, name=f"or{s}_{off}", tag="or", bufs=3)
                    nc.scalar.activation(orow[:, :], psr[:, :], COPY)
                    nc.gpsimd.dma_start(out_ap[0:1, s * cap + off:s * cap + off + n],
                                        orow[:, :])

            # segment-skewed software pipeline: L0(s+1) overlaps L1(s)/L2(s-1)
            for step in range(NSEG + 2):
                if step < NSEG:
                    load_seg(step)
                    vc_layer(step, 0, xin[step], x1)
                    xin.pop(step)
                if 1 <= step < NSEG + 1:
                    vc_layer(step - 1, 1, x1[step - 1], x2)
                    x1.pop(step - 1)
                if step >= 2:
                    head_layer(step - 2)
                    x2.pop(step - 2)

    nc.compile()
    return nc


def _prep_host(treatment, features, W0, b0, W1, b1, W2, b2):
    t = np.asarray(treatment, dtype=np.float32)
    x = np.asarray(features, dtype=np.float32)
    seg = np.searchsorted(KNOTS.astype(np.float32), t, side='right')

    # deal each segment round-robin across cores
    core_of = np.empty(B, dtype=np.int64)
    pos_of = np.empty(B, dtype=np.int64)
    counts = np.zeros((N_CORES, NSEG), dtype=np.int64)
    for m in range(NSEG):
        idx = np.nonzero(seg == m)[0]
        for c in range(N_CORES):
            sub = idx[c::N_CORES]
            core_of[sub] = c
            pos_of[sub] = np.arange(len(sub))
            counts[c, m] = len(sub)
    maxn = int(counts.max())
    cap = max(512, ((maxn + 127) // 128) * 128)
    bp = cap * NSEG

    gather = np.full((N_CORES, bp), -1, dtype=np.int64)
    slot = seg * cap + pos_of
    gather[core_of, slot] = np.arange(B)

    xT = np.zeros((N_CORES, D, bp), dtype=np.float32)
    tp = np.zeros((N_CORES, NSB, bp), dtype=np.float32)
    for c in range(N_CORES):
        v = gather[c] >= 0
        gi = gather[c][v]
        xT[c][:, v] = x[gi].T
        tv = t[gi].astype(np.float64)
        tp[c][:, v] = np.stack([tv ** p for p in range(NSB)]).astype(np.float32)

    g = _gamma()
    cw = []
    for W, b in ((W0, b0), (W1, b1)):
        Ws = np.asarray(W, dtype=np.float64).reshape(SDIM, D, D)
        c = np.einsum('mps,sio->mpio', g, Ws).reshape(NSEG, NSB * D, D)
        cb = np.einsum('mps,so->mpo', g, np.asarray(b, np.float64))
        packed = np.zeros((NSEG, 128, (NKT + 1) * D), dtype=np.float32)
        for kt in range(NKT):
            packed[:, :, kt * D:(kt + 1) * D] = c[:, kt * 128:(kt + 1) * 128, :]
        packed[:, 0:NSB, NKT * D:] = cb
        cw.append(packed)
    c2w = np.einsum('mps,si->mip', g, np.asarray(W2, np.float64))   # (9, 256, 4)
    c2b = np.einsum('mps,s->mp', g, np.asarray(b2, np.float64)[:, 0])
    c2 = np.zeros((NSEG, 128, 2 * NSB + 1), dtype=np.float32)
    for h in range(2):
        c2[:, :, h * NSB:(h + 1) * NSB] = c2w[:, h * 128:(h + 1) * 128, :]
    c2[:, 0:NSB, 2 * NSB] = c2b

    shared = dict(c0w=np.ascontiguousarray(cw[0]), c1w=np.ascontiguousarray(cw[1]),
                  c2=np.ascontiguousarray(c2),
                  ones4=np.ones((NSB, 1), np.float32))
    in_maps = [dict(shared, xT=np.ascontiguousarray(xT[c]),
                    tp=np.ascontiguousarray(tp[c])) for c in range(N_CORES)]
    return cap, in_maps, gather


def kernel(treatment, features, W0, b0, W1, b1, W2, b2):
    global LAST_EXEC_NS, LAST_MEAN_EXEC_NS, LAST_RES
    cap, in_maps, gather = _prep_host(treatment, features, W0, b0, W1, b1, W2, b2)

    if cap not in _PROG_CACHE:
        _PROG_CACHE[cap] = _build_program(cap)
    nc = _PROG_CACHE[cap]

    if TRACE:
        _register_ntff_hook()
    res = bass_utils.run_bass_kernel_spmd(
        nc, in_maps, core_ids=list(range(N_CORES)), trace=TRACE)
    LAST_EXEC_NS = res.exec_time_ns
    LAST_MEAN_EXEC_NS = res.mean_exec_time_ns
    LAST_RES = res

    out = np.empty((B,), dtype=np.float32)
    for c in range(N_CORES):
        row = res.results[c]["out"][0]
        v = gather[c] >= 0
        out[gather[c][v]] = row[v]
    return out.reshape(B, 1)
